# revision 1
# baseline (speedup 1.0000x reference)
"""Causal multi-head attention block for TRN2, data-parallel over batch.

Problem: nn_Attention (B=8, S=2048, E=768, n_heads=12, d_head=64), fp32 I/O.

Strategy: batch element b -> NeuronCore b (8 cores, no collectives).
Per core, one full causal attention layer:
  - x^T via PE transpose; Q/K/V projections packed 2 heads per matmul
    (lhsT = [W[2p] | W[2p+1]] of shape [E,128]) producing qT2/kT2/vT2
    [128=2*64 head rows, S]; vT2 is PE-transposed into v2 [s, 2*65] with a
    ones column appended per head (softmax denominator comes out of the
    z-matmul for free).
  - Attention is emitted per head-pair, fused right after that pair's
    projections. Per 512-wide q window, per k tile: the two heads' scoresT
    matmuls [128 k, 512 q] use PE row groups 0-63 / 64-127 (K=64 each) so
    they run concurrently in the array; one Exp (scale=1/8) covers both
    heads' tiles straight out of PSUM into bf16 SBUF; causal mask via
    gpsimd affine_select on the diagonal band; zT[65,512] accumulates
    v_aug^T @ expT over k tiles (row 64 = denominator).
  - Normalize: zT -> SBUF copy, DVE reciprocal, DRAM-bounce DMA partition
    broadcast, DVE multiply into zT_all (odd heads relocated to partitions
    64:128 by SBUF->SBUF DMA).
  - Output projection: out[q,e] = zT_all^T @ W_O_flat + b_O.

Compute dtype bf16 (PE fp32 runs at 1/4 rate); all accumulation fp32 in
PSUM; softmax normalization fp32.
"""
import sys

if '/opt/trn_rl_repo' not in sys.path:
    sys.path.insert(0, '/opt/trn_rl_repo')

import numpy as np
import concourse.bass as bass
import concourse.bacc as bacc
import concourse.mybir as mybir
import concourse.tile as tile
from concourse.bass_utils import run_bass_kernel_spmd
from concourse.masks import make_identity

F32 = mybir.dt.float32
BF16 = mybir.dt.bfloat16
CDT = BF16  # compute dtype for big matmuls

B, S, E = 8, 2048, 768
NH, HD = 12, 64
P = 128
ET = E // P            # 6 e-tiles
ST = S // P            # 16 s-tiles
WQ = 512               # q window
NW = S // WQ           # 4 windows
NPAIR = NH // 2        # 6 head pairs
N_CORES = 8

Exp = mybir.ActivationFunctionType.Exp
mult = mybir.AluOpType.mult
add_op = mybir.AluOpType.add
is_ge = mybir.AluOpType.is_ge


def _build(repeat=1, stage=2):
    nc = bacc.Bacc(None, target_bir_lowering=False)
    x = nc.dram_tensor("x", [S, E], F32, kind="ExternalInput")
    wq = nc.dram_tensor("wq", [NH, E, HD], F32, kind="ExternalInput")
    wk = nc.dram_tensor("wk", [NH, E, HD], F32, kind="ExternalInput")
    wv = nc.dram_tensor("wv", [NH, E, HD], F32, kind="ExternalInput")
    wo = nc.dram_tensor("wo", [NH, HD, E], F32, kind="ExternalInput")
    bq = nc.dram_tensor("bq", [NH, HD], F32, kind="ExternalInput")
    bk = nc.dram_tensor("bk", [NH, HD], F32, kind="ExternalInput")
    bv = nc.dram_tensor("bv", [NH, HD], F32, kind="ExternalInput")
    bo = nc.dram_tensor("bo", [E], F32, kind="ExternalInput")
    out = nc.dram_tensor("out", [S, E], F32, kind="ExternalOutput")
    scr = nc.dram_tensor("scr", [NH, NW, WQ], F32)  # recip bounce buffer

    with tile.TileContext(nc) as tc:
        with (
            tc.tile_pool(name="const", bufs=1) as cp,
            tc.tile_pool(name="persist", bufs=1) as pp,
        ):
            ident32 = cp.tile([P, P], F32, tag="id32")
            make_identity(nc, ident32[:])
            ident16 = cp.tile([P, P], CDT, tag="id16")
            make_identity(nc, ident16[:])
            bo_bc = cp.tile([P, E], F32, tag="bo_bc")
            nc.sync.dma_start(
                bo_bc[:],
                bo.rearrange("(one e) -> one e", one=1).to_broadcast([P, E]))

            bias_t = {}
            for name, src in (("q", bq), ("k", bk), ("v", bv)):
                flat = src.rearrange("n h -> (n h)")
                for p in range(NPAIR):
                    t = cp.tile([P, 1], F32, tag=f"b{name}2_{p}")
                    nc.sync.dma_start(
                        t[:], flat[p * P:(p + 1) * P].rearrange("(p one) -> p one", one=1))
                    bias_t[name, p] = t

            # W_O as [768, 768] flat, bf16
            wo_flat = wo.rearrange("n h e -> (n h) e")
            wo_sb = []
            for ht in range(ET):
                st_ = cp.tile([P, E], F32, tag="wo_stage")
                nc.sync.dma_start(st_[:], wo_flat[ht * P:(ht + 1) * P, :])
                t = cp.tile([P, E], CDT, tag=f"wo_{ht}")
                nc.vector.tensor_copy(t[:], st_[:])
                wo_sb.append(t)

            zT_all = [pp.tile([P, S], CDT, tag=f"zT_{p}", name=f"zT_{p}") for p in range(NPAIR)]
            if stage < 2:
                for p in range(NPAIR):
                    nc.gpsimd.memset(zT_all[p][:], 0.25)

            def _iteration():
                # ====== main: xT; software-pipelined proj/attention/outproj ======
                with (
                    tc.tile_pool(name="pm_x", bufs=3) as p_x,
                    tc.tile_pool(name="pm_xt", bufs=1) as p_xt,
                    tc.tile_pool(name="pm_qk", bufs=2) as p_qk,
                    tc.tile_pool(name="pm_v", bufs=3) as p_v,
                    tc.tile_pool(name="pm_w", bufs=3) as p_w,
                    tc.tile_pool(name="pm_wsb", bufs=3) as p_wsb,
                    tc.tile_pool(name="pm_vt", bufs=2) as p_vt,
                    tc.tile_pool(name="pm_exp", bufs=28) as p_exp,
                    tc.tile_pool(name="pm_zcp", bufs=4) as p_zcp,
                    tc.tile_pool(name="pm_bc", bufs=3) as p_bc,
                    tc.tile_pool(name="pm_zn", bufs=3) as p_zn,
                    tc.tile_pool(name="ps_a", bufs=2, space="PSUM") as ps_a,
                    tc.tile_pool(name="ps_s", bufs=2, space="PSUM") as ps_s,
                    tc.tile_pool(name="ps_z", bufs=1, space="PSUM") as ps_z,
                ):
                    # x -> xT (bf16), via PE transpose of 128x128 blocks
                    xT = [p_xt.tile([P, S], CDT, tag=f"xT_{e}", name=f"xT_{e}") for e in range(ET)]
                    for st in range(ST):
                        xs = p_x.tile([P, E], F32, tag="xs")
                        nc.sync.dma_start(xs[:], x[st * P:(st + 1) * P, :])
                        for et in range(ET):
                            tp = ps_a.tile([P, P], F32, tag="acc")
                            nc.tensor.transpose(tp[:], xs[:, et * P:(et + 1) * P], ident32[:])
                            nc.vector.tensor_copy(xT[et][:, st * P:(st + 1) * P], tp[:])

                    tiles = {}

                    def emit_proj(pr):
                        """Generator: projections for pair pr in 12 chunks."""
                        qT2 = p_qk.tile([P, S], CDT, tag="qT2", name=f"qT2_{pr}")
                        kT2 = p_qk.tile([P, S], CDT, tag="kT2", name=f"kT2_{pr}")
                        v2 = p_v.tile([P, ST, 130], CDT, tag="v2", name=f"v2_{pr}")
                        tiles[pr] = (qT2, kT2, v2)
                        v4 = v2.rearrange("q st (c d) -> q st c d", c=2, d=65)
                        nc.gpsimd.memset(v4[:, :, :, 64:65], 1.0)
                        for name, w_dram, tgt0 in (("q", wq, qT2), ("k", wk, kT2), ("v", wv, None)):
                            ws = p_w.tile([P, ET, 2 * HD], F32, tag="ws")
                            for j in range(2):
                                nc.sync.dma_start(
                                    ws[:, :, j * HD:(j + 1) * HD],
                                    w_dram[2 * pr + j].rearrange("(et q) h -> q et h", q=P))
                            wt = p_wsb.tile([P, ET, 2 * HD], CDT, tag="wt")
                            nc.vector.tensor_copy(wt[:], ws[:])
                            if name == "v":
                                vt2 = p_vt.tile([P, S], CDT, tag="vt2")
                            for sc in range(NW):
                                acc = ps_a.tile([P, WQ], F32, tag="acc")
                                for et in range(ET):
                                    nc.tensor.matmul(
                                        acc[:], wt[:, et, :], xT[et][:, sc * WQ:(sc + 1) * WQ],
                                        start=(et == 0), stop=(et == ET - 1))
                                tgt = vt2 if name == "v" else tgt0
                                nc.vector.tensor_scalar(
                                    out=tgt[:, sc * WQ:(sc + 1) * WQ], in0=acc[:],
                                    scalar1=bias_t[name, pr][:], scalar2=None,
                                    op0=add_op)
                                yield
                        # v: transpose into [s, 2*65] layout with ones cols
                        for st in range(ST):
                            tpv = ps_a.tile([P, P], CDT, tag="acc")
                            nc.tensor.transpose(
                                tpv[:], vt2[:, st * P:(st + 1) * P], ident16[:])
                            nc.vector.tensor_copy(
                                v4[:, st, :, 0:64],
                                tpv[:].rearrange("s (c d) -> s c d", c=2, d=64))
                        yield

                    def scores_exp(pr, w):
                        """Scores + exp for (pr, w); returns pending z work."""
                        qT2, kT2, v2 = tiles[pr]
                        kmax = 4 * (w + 1)
                        expts = []
                        for kt in range(kmax if stage >= 1 else 0):
                            j = kt - 4 * w
                            c0 = P * j if j > 0 else 0
                            cw = WQ - c0
                            sps = ps_s.tile([P, 2, WQ], F32, tag="sps")
                            for h2 in range(2):
                                hs = HD * h2
                                nc.tensor.matmul(
                                    sps[:, h2, c0:WQ],
                                    kT2[hs:hs + HD, kt * P:(kt + 1) * P],
                                    qT2[hs:hs + HD, w * WQ + c0:(w + 1) * WQ],
                                    start=True, stop=True)
                            expt = p_exp.tile([P, 2, WQ], CDT, tag="expt")
                            nc.scalar.activation(
                                expt[:, :, c0:WQ], sps[:, :, c0:WQ], Exp, scale=0.125)
                            if j >= 0:
                                # only cols [c0, c0+128) can have masked
                                # entries (iota = jj - p >= 0 for jj >= 128)
                                mw = min(P, cw)
                                nc.gpsimd.affine_select(
                                    out=expt[:, :, c0:c0 + mw], in_=expt[:, :, c0:c0 + mw],
                                    compare_op=is_ge, fill=0.0, base=0,
                                    channel_multiplier=-1,
                                    pattern=[[0, 2], [1, mw]])
                            expts.append((expt, c0))
                        return (pr, w, expts, kmax)

                    def z_norm(pend):
                        pr, w, expts, kmax = pend
                        if stage < 2:
                            return
                        _, _, v2 = tiles[pr]
                        zps_e = ps_z.tile([65, WQ], F32, tag="zps_e")
                        zps_o = ps_z.tile([65, WQ], F32, tag="zps_o")
                        for kt in range(kmax):
                            expt, c0 = expts[kt]
                            for h2, zps in ((0, zps_e), (1, zps_o)):
                                nc.tensor.matmul(
                                    zps[:, c0:WQ], v2[:, kt, 65 * h2:65 * h2 + 65],
                                    expt[:, h2, c0:WQ],
                                    start=(kt == 0), stop=(kt == kmax - 1))
                        for h2, zps in ((0, zps_e), (1, zps_o)):
                            n = 2 * pr + h2
                            zcp = p_zcp.tile([65, WQ], F32, tag="zcp")
                            nc.vector.tensor_copy(zcp[:], zps[:])
                            nc.vector.reciprocal(zcp[64:65, :], zps[64:65, :])
                            nc.gpsimd.dma_start(
                                scr[n, w, :].rearrange("(one q) -> one q", one=1),
                                zcp[64:65, :])
                            bc = p_bc.tile([64, WQ], F32, tag="bc")
                            nc.gpsimd.dma_start(
                                bc[:],
                                scr[n, w, :].rearrange("(one q) -> one q", one=1)
                                .to_broadcast([64, WQ]))
                            if h2 == 0:
                                nc.vector.tensor_tensor(
                                    zT_all[pr][0:64, w * WQ:(w + 1) * WQ],
                                    zcp[0:64, :], bc[:], op=mult)
                            else:
                                zn = p_zn.tile([64, WQ], CDT, tag="zn")
                                nc.vector.tensor_tensor(
                                    zn[:], zcp[0:64, :], bc[:], op=mult)
                                nc.gpsimd.dma_start(
                                    zT_all[pr][64:P, w * WQ:(w + 1) * WQ], zn[:])

                    def outproj(w):
                        for qc in range(4):
                            row0 = w * WQ + qc * P
                            for ec, ecw in ((0, 512), (1, 256)):
                                po = ps_a.tile([P, 512], F32, tag="acc")
                                for ht in range(ET):
                                    nc.tensor.matmul(
                                        po[:, 0:ecw],
                                        zT_all[ht][:, row0:row0 + P],
                                        wo_sb[ht][:, ec * 512:ec * 512 + ecw],
                                        start=(ht == 0), stop=(ht == ET - 1))
                                ost = p_zn.tile([P, 512], F32, tag="ost")
                                nc.vector.tensor_tensor(
                                    ost[:, 0:ecw], po[:, 0:ecw],
                                    bo_bc[:, ec * 512:ec * 512 + ecw], op=add_op)
                                nc.sync.dma_start(
                                    out[row0:row0 + P, ec * 512:ec * 512 + ecw],
                                    ost[:, 0:ecw])

                    # software pipeline: scores(pr,w) | z(prev) | proj(pr+1)
                    gen = emit_proj(0)
                    for _ in gen:
                        pass
                    nxt = None
                    pend = None
                    for pr in range(NPAIR):
                        if pr + 1 < NPAIR:
                            nxt = emit_proj(pr + 1)
                        for w in range(NW):
                            cur = scores_exp(pr, w)
                            if pend is not None:
                                z_norm(pend)
                            pend = cur
                            if pr + 1 < NPAIR:
                                for _ in range((2, 3, 4, 4)[w]):
                                    next(nxt, None)
                            if pr == NPAIR - 1 and w >= 1:
                                outproj(w - 1)
                    z_norm(pend)
                    outproj(NW - 1)

            for _rep in range(repeat):
                _iteration()
    nc.finalize()
    return nc


_NC_CACHE = []


def kernel(**inputs):
    xfull = np.ascontiguousarray(np.asarray(inputs["normalized_resid_pre"], np.float32))
    shared = {
        "wq": np.ascontiguousarray(np.asarray(inputs["W_Q"], np.float32)),
        "wk": np.ascontiguousarray(np.asarray(inputs["W_K"], np.float32)),
        "wv": np.ascontiguousarray(np.asarray(inputs["W_V"], np.float32)),
        "wo": np.ascontiguousarray(np.asarray(inputs["W_O"], np.float32)),
        "bq": np.ascontiguousarray(np.asarray(inputs["b_Q"], np.float32)),
        "bk": np.ascontiguousarray(np.asarray(inputs["b_K"], np.float32)),
        "bv": np.ascontiguousarray(np.asarray(inputs["b_V"], np.float32)),
        "bo": np.ascontiguousarray(np.asarray(inputs["b_O"], np.float32)),
    }
    in_maps = [{"x": xfull[c], **shared} for c in range(N_CORES)]
    if not _NC_CACHE:
        _NC_CACHE.append(_build())
    nc = _NC_CACHE[0]
    res = run_bass_kernel_spmd(nc, in_maps, core_ids=list(range(N_CORES)))
    return np.stack([res.results[c]["out"] for c in range(N_CORES)], axis=0)


if __name__ == "__main__":
    rng = np.random.default_rng(0)
    ins = {
        "normalized_resid_pre": rng.standard_normal((B, S, E), dtype=np.float32),
        "W_Q": (rng.standard_normal((NH, E, HD)) * 0.02).astype(np.float32),
        "W_K": (rng.standard_normal((NH, E, HD)) * 0.02).astype(np.float32),
        "W_V": (rng.standard_normal((NH, E, HD)) * 0.02).astype(np.float32),
        "W_O": (rng.standard_normal((NH, HD, E)) * 0.02).astype(np.float32),
        "b_Q": np.zeros((NH, HD), np.float32),
        "b_K": np.zeros((NH, HD), np.float32),
        "b_V": np.zeros((NH, HD), np.float32),
        "b_O": np.zeros((E,), np.float32),
    }
    got = kernel(**ins)
    x = ins["normalized_resid_pre"].astype(np.float64)
    q = np.einsum('bse,neh->bsnh', x, ins["W_Q"]) + ins["b_Q"]
    k = np.einsum('bse,neh->bsnh', x, ins["W_K"]) + ins["b_K"]
    v = np.einsum('bse,neh->bsnh', x, ins["W_V"]) + ins["b_V"]
    sc = np.einsum('bqnh,bknh->bnqk', q, k) / np.sqrt(64.0)
    mask = np.tril(np.ones((S, S)))
    sc = np.where(mask[None, None] > 0, sc, -1e5)
    sc = sc - sc.max(-1, keepdims=True)
    pr = np.exp(sc); pr /= pr.sum(-1, keepdims=True)
    z = np.einsum('bnqk,bknh->bqnh', pr, v)
    ref = np.einsum('bqnh,nhe->bqe', z, ins["W_O"]) + ins["b_O"]
    err = np.abs(got - ref).max() / np.abs(ref).max()
    print("rel err vs fp64 numpy:", err)



# revision 29
# speedup vs baseline: 1.0457x; 1.0457x over previous
"""Causal multi-head attention block for TRN2, data-parallel over batch.

Problem: nn_Attention (B=8, S=2048, E=768, n_heads=12, d_head=64), fp32 I/O.

Strategy: batch element b -> NeuronCore b (8 cores, no collectives).
Per core, one full causal attention layer:
  - x^T via PE transpose; kept twice: bf16 tiles (V proj) and an fp8 copy
    (Q/K proj via DoubleRow).
  - Q/K projections in fp8e4m3 DoubleRow perf mode (weights pre-scaled by
    16 to clear the subnormal range; the 16*16 factor is folded into the
    exp scale). Two heads per matmul; 2 e-tiles contracted per instruction.
  - V projection in natural [s, h] orientation (no transpose needed for
    the reoriented z matmul): v2 [s, st, 2*65] with a ones column per head
    (softmax denominator comes out of the z-matmul for free).
  - Scores per head-pair (bf16): per 512-wide q window, per k tile, the
    two heads' scoresT matmuls [128 k, 512 q] use PE row groups 0-63 /
    64-127; one Exp (scale=1/(8*256)) covers both heads straight out of
    PSUM into bf16 SBUF; causal mask via gpsimd affine_select.
  - z in [q, h] orientation: out[q-sub 128, 65] accumulated over k tiles
    with lhsT = expT block, rhs = v2 slice (65-wide moving operand =
    ~8x fewer PE cycles than the 512-wide transposed form). Softmax
    denominators land per-partition: one strided reciprocal [128, 4] and
    per-sub tensor_scalar multiplies (no partition broadcast needed).
    z_sb [q, 2*64] f32 -> PE transpose (4 batched per PSUM tile) -> zT_cat
    bf16 [128, 6, S].
  - Output projection: out[q,e] = zT_cat^T @ W_O_flat + b_O (bf16; fp8
    here fails the error budget - the max |out| element's own ~2.4%
    quantization noise is ~2.7e-2 of max).

Scheduling: the Exp stream on the Activation engine (~219us) and the
matmul stream on the PE (~229us) are the two near-equal bottlenecks, so
emission interleaves them at k-tile granularity: after each score tile,
a pump pulls ~2 chunks of deferred PE work (z-accumulation subtiles of
the previous window at priority, then projection chunks of the next
pair / the next iteration's xT build, then output-projection chunks).
Generators carry across `repeat` iterations so an iteration's tail
(last z + output projection) fills the next iteration's startup.

Compute dtype bf16 (fp8 where noted); accumulation fp32 in PSUM.
"""
import sys

if '/opt/trn_rl_repo' not in sys.path:
    sys.path.insert(0, '/opt/trn_rl_repo')

import numpy as np
import concourse.bass as bass
import concourse.bacc as bacc
import concourse.mybir as mybir
import concourse.tile as tile
from concourse.bass_utils import run_bass_kernel_spmd
from concourse.masks import make_identity

F32 = mybir.dt.float32
BF16 = mybir.dt.bfloat16
FP8 = mybir.dt.float8e4
CDT = BF16  # compute dtype for big matmuls
DR = mybir.MatmulPerfMode.DoubleRow

B, S, E = 8, 2048, 768
NH, HD = 12, 64
P = 128
ET = E // P            # 6 e-tiles
ETP = ET // 2          # 3 e-tile pairs (DoubleRow)
ST = S // P            # 16 s-tiles
WQ = 512               # q window
NW = S // WQ           # 4 windows
NPAIR = NH // 2        # 6 head pairs
N_CORES = 8

WSCALE = 16.0          # fp8 weight pre-scale (Q, K, W_O)
ZSCALE = 32.0          # fp8 z pre-scale (clears subnormals for diffuse rows)
EXP_SCALE = 0.125 / (WSCALE * WSCALE)

Exp = mybir.ActivationFunctionType.Exp
mult = mybir.AluOpType.mult
add_op = mybir.AluOpType.add
is_ge = mybir.AluOpType.is_ge


def _build(repeat=1, stage=2):
    nc = bacc.Bacc(None, target_bir_lowering=False)
    x = nc.dram_tensor("x", [S, E], F32, kind="ExternalInput")
    wq = nc.dram_tensor("wq", [NH, E, HD], F32, kind="ExternalInput")
    wk = nc.dram_tensor("wk", [NH, E, HD], F32, kind="ExternalInput")
    wv = nc.dram_tensor("wv", [NH, E, HD], F32, kind="ExternalInput")
    wo = nc.dram_tensor("wo", [NH, HD, E], F32, kind="ExternalInput")
    bq = nc.dram_tensor("bq", [NH, HD], F32, kind="ExternalInput")
    bk = nc.dram_tensor("bk", [NH, HD], F32, kind="ExternalInput")
    bv = nc.dram_tensor("bv", [NH, HD], F32, kind="ExternalInput")
    bo = nc.dram_tensor("bo", [E], F32, kind="ExternalInput")
    out = nc.dram_tensor("out", [S, E], F32, kind="ExternalOutput")

    with tile.TileContext(nc) as tc:
        with (
            tc.tile_pool(name="const", bufs=1) as cp,
            tc.tile_pool(name="persist", bufs=1) as pp,
        ):
            ident32 = cp.tile([P, P], F32, tag="id32")
            make_identity(nc, ident32[:])
            # q/k biases, scaled by WSCALE to match the scaled scores
            bias_t = {}
            for name, src in (("q", bq), ("k", bk)):
                flat = src.rearrange("n h -> (n h)")
                stg = cp.tile([P, NPAIR], F32, tag=f"b{name}_stg")
                for p in range(NPAIR):
                    nc.sync.dma_start(
                        stg[:, p:p + 1],
                        flat[p * P:(p + 1) * P].rearrange("(p one) -> p one", one=1))
                    t = cp.tile([P, 1], F32, tag=f"b{name}2_{p}")
                    nc.vector.tensor_scalar(
                        out=t[:], in0=stg[:, p:p + 1],
                        scalar1=WSCALE, scalar2=None, op0=mult)
                    bias_t[name, p] = t

            bo_bc = cp.tile([P, E], F32, tag="bo_bc")
            nc.sync.dma_start(
                bo_bc[:],
                bo.rearrange("(one e) -> one e", one=1).to_broadcast([P, E]))
            # v bias broadcast across partitions: [128, 12*64]
            bv_bc = cp.tile([P, NH * HD], F32, tag="bv_bc")
            nc.sync.dma_start(
                bv_bc[:],
                bv.rearrange("n h -> (n h)")
                .rearrange("(one e) -> one e", one=1).to_broadcast([P, NH * HD]))
            bvs = bv_bc.rearrange("p (n h) -> p n h", h=HD)

            # W_O as [768, 768] flat, bf16
            wo_flat = wo.rearrange("n h e -> (n h) e")
            wo_cat = cp.tile([P, ET, E], CDT, tag="wo_cat")
            for ht in range(ET):
                st_ = cp.tile([P, E], F32, tag="wo_stage")
                nc.sync.dma_start(st_[:], wo_flat[ht * P:(ht + 1) * P, :])
                nc.vector.tensor_copy(wo_cat[:, ht, :], st_[:])

            zT_cat = pp.tile([P, ET, S], CDT, tag="zT_cat", name="zT_cat")
            if stage < 2:
                nc.gpsimd.memset(zT_cat[:], 0.25)

            with (
                tc.tile_pool(name="pm_x", bufs=6) as p_x,
                tc.tile_pool(name="pm_xt", bufs=1) as p_xt,
                tc.tile_pool(name="pm_qk", bufs=2) as p_qk,
                tc.tile_pool(name="pm_v", bufs=3) as p_v,
                tc.tile_pool(name="pm_w", bufs=2) as p_w,
                tc.tile_pool(name="pm_wsb", bufs=2) as p_wsb,
                tc.tile_pool(name="pm_exp", bufs=28) as p_exp,
                tc.tile_pool(name="pm_zsb", bufs=6) as p_zsb,
                tc.tile_pool(name="pm_rc", bufs=4) as p_rc,
                tc.tile_pool(name="pm_zn", bufs=3) as p_zn,
                tc.tile_pool(name="ps_a", bufs=2, space="PSUM") as ps_a,
                tc.tile_pool(name="ps_s", bufs=2, space="PSUM") as ps_s,
                tc.tile_pool(name="ps_z", bufs=1, space="PSUM") as ps_z,
            ):
                tiles = {}
                prog = {}  # pr -> {"qk": windows done, "v": s-tiles done}
                xts = {}   # current iteration's xT handles (set by emit_proj(0))

                def emit_proj(pr):
                    """Generator: projections for pair pr. For pair 0 the
                    xT group builds (x -> xT bf16 + fp8, 4 transposes batched
                    per PSUM tile) and window-local v chunks are folded in so
                    attention can start after the first s-group."""
                    if pr == 0:
                        xts["xT"] = [
                            p_xt.tile([P, S], CDT, tag=f"xT_{e}", name=f"xT_{e}")
                            for e in range(ET)]
                        xts["xT8"] = p_xt.tile([P, ET, S], FP8, tag="xT8", name="xT8")
                        xts["x8v"] = xts["xT8"].rearrange("p (g i) s -> p g i s", i=2)
                    xT, xT8, x8v = xts["xT"], xts["xT8"], xts["x8v"]
                    pg = prog[pr] = {"qk": 0, "v": 0}
                        qT2 = p_qk.tile([P, S], CDT, tag="qT2", name=f"qT2_{pr}")
                        kT2 = p_qk.tile([P, S], CDT, tag="kT2", name=f"kT2_{pr}")
                        v2 = p_v.tile([P, ST, 130], CDT, tag="v2", name=f"v2_{pr}")
                        tiles[pr] = (qT2, kT2, v2)
                        v4 = v2.rearrange("q st (c d) -> q st c d", c=2, d=65)
                        nc.gpsimd.memset(v4[:, :, :, 64:65], 1.0)
                        # weight staging: q/k fp8 (scaled), v bf16
                        w8s = {}
                        tgts = {"q": qT2, "k": kT2}
                        for name, w_dram in (("q", wq), ("k", wk)):
                            ws = p_w.tile([P, ET, 2 * HD], F32, tag="ws", name=f"ws_{name}{pr}")
                            for j in range(2):
                                nc.sync.dma_start(
                                    ws[:, :, j * HD:(j + 1) * HD],
                                    w_dram[2 * pr + j].rearrange("(et p) h -> p et h", p=P))
                            w8 = p_wsb.tile([P, ET, 2 * HD], FP8, tag="w8", name=f"w8_{name}{pr}")
                            nc.vector.tensor_scalar(
                                out=w8[:], in0=ws[:], scalar1=WSCALE, scalar2=None, op0=mult)
                            w8s[name] = w8.rearrange("p (g i) m -> p g i m", i=2)
                        wsv = p_w.tile([P, ET, 2 * HD], F32, tag="ws", name=f"ws_v{pr}")
                        for j in range(2):
                            nc.sync.dma_start(
                                wsv[:, :, j * HD:(j + 1) * HD],
                                wv[2 * pr + j].rearrange("(et p) h -> p et h", p=P))
                        wtv = p_wsb.tile([P, ET, 2 * HD], CDT, tag="wtv")
                        nc.vector.tensor_copy(wtv[:], wsv[:])

                        def v_chunk(st):
                            pv = ps_a.tile([P, WQ], F32, tag="acc")
                            for et in range(ET):
                                nc.tensor.matmul(
                                    pv[:, 0:P], xT[et][:, st * P:(st + 1) * P],
                                    wtv[:, et, :],
                                    start=(et == 0), stop=(et == ET - 1))
                            nc.vector.tensor_tensor(
                                v4[:, st, :, 0:64],
                                pv[:, 0:P].rearrange("p (c d) -> p c d", c=2),
                                bvs[:, 2 * pr:2 * pr + 2, :], op=add_op)
                            pg["v"] = st + 1

                        for sc in range(NW):
                            if pr == 0 and sc > 0:
                                xs4 = [p_x.tile([P, E], F32, tag="xs", name=f"xs_{sc}_{i}")
                                       for i in range(4)]
                                for i in range(4):
                                    st = sc * 4 + i
                                    nc.sync.dma_start(xs4[i][:], x[st * P:(st + 1) * P, :])
                                for et in range(ET):
                                    tpg = ps_a.tile([P, WQ], F32, tag="acc")
                                    for i in range(4):
                                        nc.tensor.transpose(
                                            tpg[:, i * P:(i + 1) * P],
                                            xs4[i][:, et * P:(et + 1) * P], ident32[:])
                                    nc.vector.tensor_copy(
                                        xT[et][:, sc * WQ:(sc + 1) * WQ], tpg[:])
                                    nc.gpsimd.tensor_copy(
                                        xT8[:, et, sc * WQ:(sc + 1) * WQ],
                                        xT[et][:, sc * WQ:(sc + 1) * WQ])
                                    yield
                            for name in ("q", "k"):
                                acc = ps_a.tile([P, WQ], F32, tag="acc")
                                for g in range(ETP):
                                    nc.tensor.matmul(
                                        acc[:], w8s[name][:, g],
                                        x8v[:, g, :, sc * WQ:(sc + 1) * WQ],
                                        start=(g == 0), stop=(g == ETP - 1),
                                        perf_mode=DR)
                                nc.vector.tensor_scalar(
                                    out=tgts[name][:, sc * WQ:(sc + 1) * WQ], in0=acc[:],
                                    scalar1=bias_t[name, pr][:], scalar2=None,
                                    op0=add_op)
                                if name == "k":
                                    pg["qk"] = sc + 1
                                yield
                            if pr == 0:
                                for st in range(4 * sc, 4 * sc + 4):
                                    v_chunk(st)
                                    yield
                        if pr != 0:
                            for st in range(ST):
                                v_chunk(st)
                                yield

                    def scores_tile(pr, w, kt, expts):
                        """One k-tile of scoresT + exp for (pr, w)."""
                        qT2, kT2, v2 = tiles[pr]
                        j = kt - 4 * w
                        c0 = P * j if j > 0 else 0
                        cw = WQ - c0
                        sps = ps_s.tile([P, 2, WQ], F32, tag="sps")
                        for h2 in range(2):
                            hs = HD * h2
                            nc.tensor.matmul(
                                sps[:, h2, c0:WQ],
                                kT2[hs:hs + HD, kt * P:(kt + 1) * P],
                                qT2[hs:hs + HD, w * WQ + c0:(w + 1) * WQ],
                                start=True, stop=True)
                        expt = p_exp.tile([P, 2, WQ], CDT, tag="expt")
                        nc.scalar.activation(
                            expt[:, :, c0:WQ], sps[:, :, c0:WQ], Exp, scale=EXP_SCALE)
                        if j >= 0:
                            # only cols [c0, c0+128) can have masked
                            # entries (iota = jj - p >= 0 for jj >= 128)
                            mw = min(P, cw)
                            nc.gpsimd.affine_select(
                                out=expt[:, :, c0:c0 + mw], in_=expt[:, :, c0:c0 + mw],
                                compare_op=is_ge, fill=0.0, base=0,
                                channel_multiplier=-1,
                                pattern=[[0, 2], [1, mw]])
                        expts.append((expt, c0))

                    def z_gen(pend):
                        """Generator: z matmuls + normalize for (pr, w)."""
                        pr, w, expts = pend
                        if stage < 2:
                            return
                        _, _, v2 = tiles[pr]
                        rc = p_rc.tile([P, 2, 4, 1], F32, tag="rc")
                        zsb = [p_zsb.tile([P, P], F32, tag="zsb", name=f"zsb{i}") for i in range(4)]
                        for h2 in range(2):
                            zps = ps_z.tile([P, 4, 65], F32, tag=f"zps{h2}")
                            for sub in range(4):
                                n_kt = 4 * w + sub + 1
                                for kt in range(n_kt):
                                    expt, _ = expts[kt]
                                    nc.tensor.matmul(
                                        zps[:, sub, :],
                                        expt[:, h2, sub * P:(sub + 1) * P],
                                        v2[:, kt, 65 * h2:65 * h2 + 65],
                                        start=(kt == 0), stop=(kt == n_kt - 1))
                                yield
                            nc.vector.reciprocal(rc[:, h2], zps[:, :, 64:65])
                            for sub in range(4):
                                nc.vector.tensor_scalar(
                                    out=zsb[sub][:, h2 * HD:(h2 + 1) * HD],
                                    in0=zps[:, sub, 0:HD],
                                    scalar1=rc[:, h2, sub], scalar2=None, op0=mult)
                        yield
                        tpz = ps_a.tile([P, WQ], F32, tag="acc")
                        for sub in range(4):
                            nc.tensor.transpose(
                                tpz[:, sub * P:(sub + 1) * P], zsb[sub][:], ident32[:])
                        nc.vector.tensor_copy(
                            zT_all[pr][:, w * WQ:(w + 1) * WQ], tpz[:])

                    def outproj_gen(w):
                        for qc in range(4):
                            row0 = w * WQ + qc * P
                            for ec in range(3):
                                e0, ecw = ec * 256, 256
                                po = ps_a.tile([P, WQ], F32, tag="acc")
                                for ht in range(ET):
                                    nc.tensor.matmul(
                                        po[:, 0:ecw],
                                        zT_all[ht][:, row0:row0 + P],
                                        wo_sb[ht][:, e0:e0 + ecw],
                                        start=(ht == 0), stop=(ht == ET - 1))
                                ost = p_zn.tile([P, 256], F32, tag="ost")
                                nc.vector.tensor_tensor(
                                    ost[:, 0:ecw], po[:, 0:ecw],
                                    bo_bc[:, e0:e0 + ecw], op=add_op)
                                nc.sync.dma_start(
                                    out[row0:row0 + P, e0:e0 + ecw],
                                    ost[:, 0:ecw])
                                yield

                    # software pipeline: between score tiles (ACT-bound at
                    # ~2x the PE cost), pump filler PE work: z chunks of the
                    # previous window (priority), projection chunks for the
                    # next pair, and output-projection chunks.
                    zq = []      # priority queue of z generators
                    bq_ = []     # background: proj + outproj generators

                    def pump(n):
                        while n > 0:
                            q = zq if zq else bq_
                            if not q:
                                return
                            try:
                                next(q[0])
                                n -= 1
                            except StopIteration:
                                q.pop(0)

                    def ensure(fn):
                        # emission-order safety: force-drain background chunks
                        # until the producer progress condition holds
                        while not fn():
                            if not bq_:
                                raise RuntimeError("pipeline underflow")
                            try:
                                next(bq_[0])
                            except StopIteration:
                                bq_.pop(0)

                    # startup: s-group 0 + pair-0 window-0 chunks inline;
                    # everything else (xT groups 1-3, later windows, v) is
                    # pumped as filler between score tiles. The previous
                    # iteration's tail (last z + output projections) rides
                    # along as background filler instead of serializing.
                    emit_xt_group(0)
                    f0 = emit_proj(0)
                    for _ in range(6):   # q w0, k w0, v st0-3
                        next(f0)
                    zq.extend(carry[0])
                    bq_.append(f0)
                    bq_.extend(carry[1])
                    pend = None
                    for pr in range(NPAIR):
                        if pr + 1 < NPAIR:
                            bq_.append(emit_proj(pr + 1))
                        for w in range(NW):
                            if pend is not None:
                                ppr, pw, _ = pend
                                ensure(lambda: prog.get(ppr, {}).get("v", 0)
                                       >= min(4 * pw + 4, ST))
                                zq.append(z_gen(pend))
                            ensure(lambda: prog.get(pr, {}).get("qk", 0) >= w + 1)
                            expts = []
                            pend = (pr, w, expts)
                            for kt in range(4 * (w + 1) if stage >= 1 else 0):
                                scores_tile(pr, w, kt, expts)
                                pump(2)
                            if pr == NPAIR - 1 and w >= 1:
                                bq_.append(outproj_gen(w - 1))
                    ppr, pw, _ = pend
                    ensure(lambda: prog.get(ppr, {}).get("v", 0) >= ST)
                    return (zq + [z_gen(pend)], bq_ + [outproj_gen(NW - 1)])

                carry = ([], [])
                for _rep in range(repeat):
                    carry = _iteration(carry)
                for g in carry[0] + carry[1]:
                    for _ in g:
                        pass
    nc.finalize()
    return nc


_NC_CACHE = []


def kernel(**inputs):
    xfull = np.ascontiguousarray(np.asarray(inputs["normalized_resid_pre"], np.float32))
    shared = {
        "wq": np.ascontiguousarray(np.asarray(inputs["W_Q"], np.float32)),
        "wk": np.ascontiguousarray(np.asarray(inputs["W_K"], np.float32)),
        "wv": np.ascontiguousarray(np.asarray(inputs["W_V"], np.float32)),
        "wo": np.ascontiguousarray(np.asarray(inputs["W_O"], np.float32)),
        "bq": np.ascontiguousarray(np.asarray(inputs["b_Q"], np.float32)),
        "bk": np.ascontiguousarray(np.asarray(inputs["b_K"], np.float32)),
        "bv": np.ascontiguousarray(np.asarray(inputs["b_V"], np.float32)),
        "bo": np.ascontiguousarray(np.asarray(inputs["b_O"], np.float32)),
    }
    in_maps = [{"x": xfull[c], **shared} for c in range(N_CORES)]
    if not _NC_CACHE:
        _NC_CACHE.append(_build())
    nc = _NC_CACHE[0]
    res = run_bass_kernel_spmd(nc, in_maps, core_ids=list(range(N_CORES)))
    return np.stack([res.results[c]["out"] for c in range(N_CORES)], axis=0)


if __name__ == "__main__":
    rng = np.random.default_rng(0)
    ins = {
        "normalized_resid_pre": rng.standard_normal((B, S, E), dtype=np.float32),
        "W_Q": (rng.standard_normal((NH, E, HD)) * 0.02).astype(np.float32),
        "W_K": (rng.standard_normal((NH, E, HD)) * 0.02).astype(np.float32),
        "W_V": (rng.standard_normal((NH, E, HD)) * 0.02).astype(np.float32),
        "W_O": (rng.standard_normal((NH, HD, E)) * 0.02).astype(np.float32),
        "b_Q": np.zeros((NH, HD), np.float32),
        "b_K": np.zeros((NH, HD), np.float32),
        "b_V": np.zeros((NH, HD), np.float32),
        "b_O": np.zeros((E,), np.float32),
    }
    got = kernel(**ins)
    x = ins["normalized_resid_pre"].astype(np.float64)
    q = np.einsum('bse,neh->bsnh', x, ins["W_Q"]) + ins["b_Q"]
    k = np.einsum('bse,neh->bsnh', x, ins["W_K"]) + ins["b_K"]
    v = np.einsum('bse,neh->bsnh', x, ins["W_V"]) + ins["b_V"]
    sc = np.einsum('bqnh,bknh->bnqk', q, k) / np.sqrt(64.0)
    mask = np.tril(np.ones((S, S)))
    sc = np.where(mask[None, None] > 0, sc, -1e5)
    sc = sc - sc.max(-1, keepdims=True)
    pr = np.exp(sc); pr /= pr.sum(-1, keepdims=True)
    z = np.einsum('bnqk,bknh->bqnh', pr, v)
    ref = np.einsum('bqnh,nhe->bqe', z, ins["W_O"]) + ins["b_O"]
    err = np.abs(got - ref).max() / np.abs(ref).max()
    print("rel err vs fp64 numpy:", err)


# revision 32
# speedup vs baseline: 1.1098x; 1.0612x over previous
"""Causal multi-head attention block for TRN2, data-parallel over batch.

Problem: nn_Attention (B=8, S=2048, E=768, n_heads=12, d_head=64), fp32 I/O.

Strategy: batch element b -> NeuronCore b (8 cores, no collectives).
Per core, one full causal attention layer:
  - x^T via PE transpose; kept twice: bf16 tiles (V proj) and an fp8 copy
    (Q/K proj via DoubleRow).
  - Q/K projections in fp8e4m3 DoubleRow perf mode (weights pre-scaled by
    16 to clear the subnormal range; the 16*16 factor is folded into the
    exp scale). Two heads per matmul; 2 e-tiles contracted per instruction.
  - V projection in natural [s, h] orientation (no transpose needed for
    the reoriented z matmul): v2 [s, st, 2*65] with a ones column per head
    (softmax denominator comes out of the z-matmul for free).
  - Scores per head-pair (bf16): per 512-wide q window, per k tile, the
    two heads' scoresT matmuls [128 k, 512 q] use PE row groups 0-63 /
    64-127; one Exp (scale=1/(8*256)) covers both heads straight out of
    PSUM into bf16 SBUF; causal mask via gpsimd affine_select.
  - z in [q, h] orientation: out[q-sub 128, 65] accumulated over k tiles
    with lhsT = expT block, rhs = v2 slice (65-wide moving operand =
    ~8x fewer PE cycles than the 512-wide transposed form). Softmax
    denominators land per-partition: one strided reciprocal [128, 4] and
    per-sub tensor_scalar multiplies (no partition broadcast needed).
    z_sb [q, 2*64] f32 -> PE transpose (4 batched per PSUM tile) -> zT_cat
    bf16 [128, 6, S].
  - Output projection: out[q,e] = zT_cat^T @ W_O_flat + b_O (bf16; fp8
    here fails the error budget - the max |out| element's own ~2.4%
    quantization noise is ~2.7e-2 of max).

Scheduling: the Exp stream on the Activation engine (~219us) and the
matmul stream on the PE (~229us) are the two near-equal bottlenecks, so
emission interleaves them at k-tile granularity: after each score tile,
a pump pulls ~2 chunks of deferred PE work (z-accumulation subtiles of
the previous window at priority, then projection chunks of the next
pair / the next iteration's xT build, then output-projection chunks).
Generators carry across `repeat` iterations so an iteration's tail
(last z + output projection) fills the next iteration's startup.

Compute dtype bf16 (fp8 where noted); accumulation fp32 in PSUM.
"""
import sys

if '/opt/trn_rl_repo' not in sys.path:
    sys.path.insert(0, '/opt/trn_rl_repo')

import numpy as np
import concourse.bass as bass
import concourse.bacc as bacc
import concourse.mybir as mybir
import concourse.tile as tile
from concourse.bass_utils import run_bass_kernel_spmd
from concourse.masks import make_identity

F32 = mybir.dt.float32
BF16 = mybir.dt.bfloat16
FP8 = mybir.dt.float8e4
CDT = BF16  # compute dtype for big matmuls
DR = mybir.MatmulPerfMode.DoubleRow

B, S, E = 8, 2048, 768
NH, HD = 12, 64
P = 128
ET = E // P            # 6 e-tiles
ETP = ET // 2          # 3 e-tile pairs (DoubleRow)
ST = S // P            # 16 s-tiles
WQ = 512               # q window
NW = S // WQ           # 4 windows
NPAIR = NH // 2        # 6 head pairs
N_CORES = 8

WSCALE = 16.0          # fp8 weight pre-scale (Q, K, W_O)
ZSCALE = 32.0          # fp8 z pre-scale (clears subnormals for diffuse rows)
EXP_SCALE = 0.125 / (WSCALE * WSCALE)

Exp = mybir.ActivationFunctionType.Exp
mult = mybir.AluOpType.mult
add_op = mybir.AluOpType.add
is_ge = mybir.AluOpType.is_ge


def _build(repeat=1, stage=2):
    nc = bacc.Bacc(None, target_bir_lowering=False)
    x = nc.dram_tensor("x", [S, E], F32, kind="ExternalInput")
    wq = nc.dram_tensor("wq", [NH, E, HD], F32, kind="ExternalInput")
    wk = nc.dram_tensor("wk", [NH, E, HD], F32, kind="ExternalInput")
    wv = nc.dram_tensor("wv", [NH, E, HD], F32, kind="ExternalInput")
    wo = nc.dram_tensor("wo", [NH, HD, E], F32, kind="ExternalInput")
    bq = nc.dram_tensor("bq", [NH, HD], F32, kind="ExternalInput")
    bk = nc.dram_tensor("bk", [NH, HD], F32, kind="ExternalInput")
    bv = nc.dram_tensor("bv", [NH, HD], F32, kind="ExternalInput")
    bo = nc.dram_tensor("bo", [E], F32, kind="ExternalInput")
    out = nc.dram_tensor("out", [S, E], F32, kind="ExternalOutput")

    with tile.TileContext(nc) as tc:
        with (
            tc.tile_pool(name="const", bufs=1) as cp,
            tc.tile_pool(name="persist", bufs=1) as pp,
        ):
            ident32 = cp.tile([P, P], F32, tag="id32")
            make_identity(nc, ident32[:])
            # q/k biases, scaled by WSCALE to match the scaled scores
            bias_t = {}
            for name, src in (("q", bq), ("k", bk)):
                flat = src.rearrange("n h -> (n h)")
                stg = cp.tile([P, NPAIR], F32, tag=f"b{name}_stg")
                for p in range(NPAIR):
                    nc.sync.dma_start(
                        stg[:, p:p + 1],
                        flat[p * P:(p + 1) * P].rearrange("(p one) -> p one", one=1))
                    t = cp.tile([P, 1], F32, tag=f"b{name}2_{p}")
                    nc.vector.tensor_scalar(
                        out=t[:], in0=stg[:, p:p + 1],
                        scalar1=WSCALE, scalar2=None, op0=mult)
                    bias_t[name, p] = t

            bo_bc = cp.tile([P, E], F32, tag="bo_bc")
            nc.sync.dma_start(
                bo_bc[:],
                bo.rearrange("(one e) -> one e", one=1).to_broadcast([P, E]))
            # v bias broadcast across partitions: [128, 12*64]
            bv_bc = cp.tile([P, NH * HD], F32, tag="bv_bc")
            nc.sync.dma_start(
                bv_bc[:],
                bv.rearrange("n h -> (n h)")
                .rearrange("(one e) -> one e", one=1).to_broadcast([P, NH * HD]))
            bvs = bv_bc.rearrange("p (n h) -> p n h", h=HD)

            # W_O as [768, 768] flat, bf16
            wo_flat = wo.rearrange("n h e -> (n h) e")
            wo_cat = cp.tile([P, ET, E], CDT, tag="wo_cat")
            for ht in range(ET):
                st_ = cp.tile([P, E], F32, tag="wo_stage")
                nc.sync.dma_start(st_[:], wo_flat[ht * P:(ht + 1) * P, :])
                nc.vector.tensor_copy(wo_cat[:, ht, :], st_[:])

            zT_cat = pp.tile([P, ET, S], CDT, tag="zT_cat", name="zT_cat")
            if stage < 2:
                nc.gpsimd.memset(zT_cat[:], 0.25)

            with (
                tc.tile_pool(name="pm_x", bufs=6) as p_x,
                tc.tile_pool(name="pm_xt", bufs=1) as p_xt,
                tc.tile_pool(name="pm_qk", bufs=2) as p_qk,
                tc.tile_pool(name="pm_v", bufs=3) as p_v,
                tc.tile_pool(name="pm_w", bufs=2) as p_w,
                tc.tile_pool(name="pm_wsb", bufs=2) as p_wsb,
                tc.tile_pool(name="pm_exp", bufs=28) as p_exp,
                tc.tile_pool(name="pm_zsb", bufs=6) as p_zsb,
                tc.tile_pool(name="pm_rc", bufs=4) as p_rc,
                tc.tile_pool(name="pm_zn", bufs=3) as p_zn,
                tc.tile_pool(name="ps_a", bufs=2, space="PSUM") as ps_a,
                tc.tile_pool(name="ps_s", bufs=2, space="PSUM") as ps_s,
                tc.tile_pool(name="ps_z", bufs=1, space="PSUM") as ps_z,
            ):
                tiles = {}
                prog = {}  # pr -> {"qk": windows done, "v": s-tiles done}
                xts = {}   # current iteration's xT handles (set by emit_proj(0))

                def emit_proj(pr):
                    """Generator: projections for pair pr. For pair 0 the
                    xT group builds (x -> xT bf16 + fp8, 4 transposes batched
                    per PSUM tile) and window-local v chunks are folded in so
                    attention can start after the first s-group."""
                    if pr == 0:
                        xts["xT"] = [
                            p_xt.tile([P, S], CDT, tag=f"xT_{e}", name=f"xT_{e}")
                            for e in range(ET)]
                        xts["xT8"] = p_xt.tile([P, ET, S], FP8, tag="xT8", name="xT8")
                        xts["x8v"] = xts["xT8"].rearrange("p (g i) s -> p g i s", i=2)
                    xT, xT8, x8v = xts["xT"], xts["xT8"], xts["x8v"]
                    pg = prog[pr] = {"qk": 0, "v": 0}
                        qT2 = p_qk.tile([P, S], CDT, tag="qT2", name=f"qT2_{pr}")
                        kT2 = p_qk.tile([P, S], CDT, tag="kT2", name=f"kT2_{pr}")
                        v2 = p_v.tile([P, ST, 130], CDT, tag="v2", name=f"v2_{pr}")
                        tiles[pr] = (qT2, kT2, v2)
                        v4 = v2.rearrange("q st (c d) -> q st c d", c=2, d=65)
                        nc.gpsimd.memset(v4[:, :, :, 64:65], 1.0)
                        # weight staging: q/k fp8 (scaled), v bf16
                        w8s = {}
                        tgts = {"q": qT2, "k": kT2}
                        for name, w_dram in (("q", wq), ("k", wk)):
                            ws = p_w.tile([P, ET, 2 * HD], F32, tag="ws", name=f"ws_{name}{pr}")
                            for j in range(2):
                                nc.sync.dma_start(
                                    ws[:, :, j * HD:(j + 1) * HD],
                                    w_dram[2 * pr + j].rearrange("(et p) h -> p et h", p=P))
                            w8 = p_wsb.tile([P, ET, 2 * HD], FP8, tag="w8", name=f"w8_{name}{pr}")
                            nc.vector.tensor_scalar(
                                out=w8[:], in0=ws[:], scalar1=WSCALE, scalar2=None, op0=mult)
                            w8s[name] = w8.rearrange("p (g i) m -> p g i m", i=2)
                        wsv = p_w.tile([P, ET, 2 * HD], F32, tag="ws", name=f"ws_v{pr}")
                        for j in range(2):
                            nc.sync.dma_start(
                                wsv[:, :, j * HD:(j + 1) * HD],
                                wv[2 * pr + j].rearrange("(et p) h -> p et h", p=P))
                        wtv = p_wsb.tile([P, ET, 2 * HD], CDT, tag="wtv")
                        nc.vector.tensor_copy(wtv[:], wsv[:])

                        def v_chunk(st):
                            pv = ps_a.tile([P, WQ], F32, tag="acc")
                            for et in range(ET):
                                nc.tensor.matmul(
                                    pv[:, 0:P], xT[et][:, st * P:(st + 1) * P],
                                    wtv[:, et, :],
                                    start=(et == 0), stop=(et == ET - 1))
                            nc.vector.tensor_tensor(
                                v4[:, st, :, 0:64],
                                pv[:, 0:P].rearrange("p (c d) -> p c d", c=2),
                                bvs[:, 2 * pr:2 * pr + 2, :], op=add_op)
                            pg["v"] = st + 1

                        for sc in range(NW):
                            if pr == 0 and sc > 0:
                                xs4 = [p_x.tile([P, E], F32, tag="xs", name=f"xs_{sc}_{i}")
                                       for i in range(4)]
                                for i in range(4):
                                    st = sc * 4 + i
                                    nc.sync.dma_start(xs4[i][:], x[st * P:(st + 1) * P, :])
                                for et in range(ET):
                                    tpg = ps_a.tile([P, WQ], F32, tag="acc")
                                    for i in range(4):
                                        nc.tensor.transpose(
                                            tpg[:, i * P:(i + 1) * P],
                                            xs4[i][:, et * P:(et + 1) * P], ident32[:])
                                    nc.vector.tensor_copy(
                                        xT[et][:, sc * WQ:(sc + 1) * WQ], tpg[:])
                                    nc.gpsimd.tensor_copy(
                                        xT8[:, et, sc * WQ:(sc + 1) * WQ],
                                        xT[et][:, sc * WQ:(sc + 1) * WQ])
                                    yield
                            for name in ("q", "k"):
                                acc = ps_a.tile([P, WQ], F32, tag="acc")
                                for g in range(ETP):
                                    nc.tensor.matmul(
                                        acc[:], w8s[name][:, g],
                                        x8v[:, g, :, sc * WQ:(sc + 1) * WQ],
                                        start=(g == 0), stop=(g == ETP - 1),
                                        perf_mode=DR)
                                nc.vector.tensor_scalar(
                                    out=tgts[name][:, sc * WQ:(sc + 1) * WQ], in0=acc[:],
                                    scalar1=bias_t[name, pr][:], scalar2=None,
                                    op0=add_op)
                                if name == "k":
                                    pg["qk"] = sc + 1
                                yield
                            if pr == 0:
                                for st in range(4 * sc, 4 * sc + 4):
                                    v_chunk(st)
                                    yield
                        if pr != 0:
                            for st in range(ST):
                                v_chunk(st)
                                yield

                    def scores_tile(pr, w, kt, expts):
                        """One k-tile of scoresT + exp for (pr, w)."""
                        qT2, kT2, v2 = tiles[pr]
                        j = kt - 4 * w
                        c0 = P * j if j > 0 else 0
                        cw = WQ - c0
                        sps = ps_s.tile([P, 2, WQ], F32, tag="sps")
                        for h2 in range(2):
                            hs = HD * h2
                            nc.tensor.matmul(
                                sps[:, h2, c0:WQ],
                                kT2[hs:hs + HD, kt * P:(kt + 1) * P],
                                qT2[hs:hs + HD, w * WQ + c0:(w + 1) * WQ],
                                start=True, stop=True)
                        expt = p_exp.tile([P, 2, WQ], CDT, tag="expt")
                        nc.scalar.activation(
                            expt[:, :, c0:WQ], sps[:, :, c0:WQ], Exp, scale=EXP_SCALE)
                        if j >= 0:
                            # only cols [c0, c0+128) can have masked
                            # entries (iota = jj - p >= 0 for jj >= 128)
                            mw = min(P, cw)
                            nc.gpsimd.affine_select(
                                out=expt[:, :, c0:c0 + mw], in_=expt[:, :, c0:c0 + mw],
                                compare_op=is_ge, fill=0.0, base=0,
                                channel_multiplier=-1,
                                pattern=[[0, 2], [1, mw]])
                        expts.append((expt, c0))

                    def z_gen(pend):
                        """Generator: z matmuls + normalize for (pr, w)."""
                        pr, w, expts = pend
                        if stage < 2:
                            return
                        _, _, v2 = tiles[pr]
                        rc = p_rc.tile([P, 2, 4, 1], F32, tag="rc")
                        zsb = [p_zsb.tile([P, P], F32, tag="zsb", name=f"zsb{i}") for i in range(4)]
                        for h2 in range(2):
                            zps = ps_z.tile([P, 4, 65], F32, tag=f"zps{h2}")
                            for sub in range(4):
                                n_kt = 4 * w + sub + 1
                                for kt in range(n_kt):
                                    expt, _ = expts[kt]
                                    nc.tensor.matmul(
                                        zps[:, sub, :],
                                        expt[:, h2, sub * P:(sub + 1) * P],
                                        v2[:, kt, 65 * h2:65 * h2 + 65],
                                        start=(kt == 0), stop=(kt == n_kt - 1))
                                yield
                            nc.vector.reciprocal(rc[:, h2], zps[:, :, 64:65])
                            for sub in range(4):
                                nc.vector.tensor_scalar(
                                    out=zsb[sub][:, h2 * HD:(h2 + 1) * HD],
                                    in0=zps[:, sub, 0:HD],
                                    scalar1=rc[:, h2, sub], scalar2=None, op0=mult)
                        yield
                        tpz = ps_a.tile([P, WQ], F32, tag="acc")
                        for sub in range(4):
                            nc.tensor.transpose(
                                tpz[:, sub * P:(sub + 1) * P], zsb[sub][:], ident32[:])
                        nc.vector.tensor_copy(
                            zT_all[pr][:, w * WQ:(w + 1) * WQ], tpz[:])

                    def outproj_gen(w):
                        for qc in range(4):
                            row0 = w * WQ + qc * P
                            for ec in range(3):
                                e0, ecw = ec * 256, 256
                                po = ps_a.tile([P, WQ], F32, tag="acc")
                                for ht in range(ET):
                                    nc.tensor.matmul(
                                        po[:, 0:ecw],
                                        zT_all[ht][:, row0:row0 + P],
                                        wo_sb[ht][:, e0:e0 + ecw],
                                        start=(ht == 0), stop=(ht == ET - 1))
                                ost = p_zn.tile([P, 256], F32, tag="ost")
                                nc.vector.tensor_tensor(
                                    ost[:, 0:ecw], po[:, 0:ecw],
                                    bo_bc[:, e0:e0 + ecw], op=add_op)
                                nc.sync.dma_start(
                                    out[row0:row0 + P, e0:e0 + ecw],
                                    ost[:, 0:ecw])
                                yield

                    # software pipeline: between score tiles (ACT-bound at
                    # ~2x the PE cost), pump filler PE work: z chunks of the
                    # previous window (priority), projection chunks for the
                    # next pair, and output-projection chunks.
                    zq = []      # priority queue of z generators
                    bq_ = []     # background: proj + outproj generators

                    def pump(n):
                        while n > 0:
                            q = zq if zq else bq_
                            if not q:
                                return
                            try:
                                next(q[0])
                                n -= 1
                            except StopIteration:
                                q.pop(0)

                    def ensure(fn):
                        # emission-order safety: force-drain background chunks
                        # until the producer progress condition holds
                        while not fn():
                            if not bq_:
                                raise RuntimeError("pipeline underflow")
                            try:
                                next(bq_[0])
                            except StopIteration:
                                bq_.pop(0)

                    # startup: s-group 0 + pair-0 window-0 chunks inline;
                    # everything else (xT groups 1-3, later windows, v) is
                    # pumped as filler between score tiles. The previous
                    # iteration's tail (last z + output projections) rides
                    # along as background filler instead of serializing.
                    emit_xt_group(0)
                    f0 = emit_proj(0)
                    for _ in range(6):   # q w0, k w0, v st0-3
                        next(f0)
                    zq.extend(carry[0])
                    bq_.append(f0)
                    bq_.extend(carry[1])
                    pend = None
                    for pr in range(NPAIR):
                        if pr + 1 < NPAIR:
                            bq_.append(emit_proj(pr + 1))
                        for w in range(NW):
                            if pend is not None:
                                ppr, pw, _ = pend
                                ensure(lambda: prog.get(ppr, {}).get("v", 0)
                                       >= min(4 * pw + 4, ST))
                                zq.append(z_gen(pend))
                            ensure(lambda: prog.get(pr, {}).get("qk", 0) >= w + 1)
                            expts = []
                            pend = (pr, w, expts)
                            for kt in range(4 * (w + 1) if stage >= 1 else 0):
                                scores_tile(pr, w, kt, expts)
                                pump(2)
                            if pr == NPAIR - 1 and w >= 1:
                                bq_.append(outproj_gen(w - 1))
                    ppr, pw, _ = pend
                    ensure(lambda: prog.get(ppr, {}).get("v", 0) >= ST)
                    return (zq + [z_gen(pend)], bq_ + [outproj_gen(NW - 1)])

                carry = ([], [])
                for _rep in range(repeat):
                    carry = _iteration(carry)
                for g in carry[0] + carry[1]:
                    for _ in g:
                        pass
    nc.finalize()
    return nc


_NC_CACHE = []


def kernel(**inputs):
    xfull = np.ascontiguousarray(np.asarray(inputs["normalized_resid_pre"], np.float32))
    shared = {
        "wq": np.ascontiguousarray(np.asarray(inputs["W_Q"], np.float32)),
        "wk": np.ascontiguousarray(np.asarray(inputs["W_K"], np.float32)),
        "wv": np.ascontiguousarray(np.asarray(inputs["W_V"], np.float32)),
        "wo": np.ascontiguousarray(np.asarray(inputs["W_O"], np.float32)),
        "bq": np.ascontiguousarray(np.asarray(inputs["b_Q"], np.float32)),
        "bk": np.ascontiguousarray(np.asarray(inputs["b_K"], np.float32)),
        "bv": np.ascontiguousarray(np.asarray(inputs["b_V"], np.float32)),
        "bo": np.ascontiguousarray(np.asarray(inputs["b_O"], np.float32)),
    }
    in_maps = [{"x": xfull[c], **shared} for c in range(N_CORES)]
    if not _NC_CACHE:
        _NC_CACHE.append(_build())
    nc = _NC_CACHE[0]
    res = run_bass_kernel_spmd(nc, in_maps, core_ids=list(range(N_CORES)))
    return np.stack([res.results[c]["out"] for c in range(N_CORES)], axis=0)


if __name__ == "__main__":
    rng = np.random.default_rng(0)
    ins = {
        "normalized_resid_pre": rng.standard_normal((B, S, E), dtype=np.float32),
        "W_Q": (rng.standard_normal((NH, E, HD)) * 0.02).astype(np.float32),
        "W_K": (rng.standard_normal((NH, E, HD)) * 0.02).astype(np.float32),
        "W_V": (rng.standard_normal((NH, E, HD)) * 0.02).astype(np.float32),
        "W_O": (rng.standard_normal((NH, HD, E)) * 0.02).astype(np.float32),
        "b_Q": np.zeros((NH, HD), np.float32),
        "b_K": np.zeros((NH, HD), np.float32),
        "b_V": np.zeros((NH, HD), np.float32),
        "b_O": np.zeros((E,), np.float32),
    }
    got = kernel(**ins)
    x = ins["normalized_resid_pre"].astype(np.float64)
    q = np.einsum('bse,neh->bsnh', x, ins["W_Q"]) + ins["b_Q"]
    k = np.einsum('bse,neh->bsnh', x, ins["W_K"]) + ins["b_K"]
    v = np.einsum('bse,neh->bsnh', x, ins["W_V"]) + ins["b_V"]
    sc = np.einsum('bqnh,bknh->bnqk', q, k) / np.sqrt(64.0)
    mask = np.tril(np.ones((S, S)))
    sc = np.where(mask[None, None] > 0, sc, -1e5)
    sc = sc - sc.max(-1, keepdims=True)
    pr = np.exp(sc); pr /= pr.sum(-1, keepdims=True)
    z = np.einsum('bnqk,bknh->bqnh', pr, v)
    ref = np.einsum('bqnh,nhe->bqe', z, ins["W_O"]) + ins["b_O"]
    err = np.abs(got - ref).max() / np.abs(ref).max()
    print("rel err vs fp64 numpy:", err)
